# revision 20
# baseline (speedup 1.0000x reference)
"""Bayesian linear layer (local reparameterization) on 8 Trainium2 NeuronCores.

Computes, for x:[8192,1024], w_mean:[1024,1024], w_pho:[1024,1024] (all f32):
    w_var = softplus(w_pho)**2
    gamma = x @ w_mean.T
    delta = (x*x) @ w_var.T
    out   = gamma + sqrt(delta) * zeta        zeta = jax.random.normal(key(42))

Sharding: data-parallel over the 8192 rows of x (1024 rows per core);
w_mean/w_pho replicated. zeta is a deterministic constant, computed host-side
with the same jax call as the reference and sharded alongside x.

Device kernel (per core):
    - host pre-transposes x and w so the contraction dim (in_features) lands on
      SBUF partitions; matmuls run as fp32r (TF32 mode: full PE rate at output
      free dim >= 256, vs 4 cycles/row for plain fp32).
    - softplus via exp -> ln(1+.) and sqrt via exp(0.5*ln(.)) on ScalarE, all
      inside one pre-loaded ACT LUT set (avoids per-op table reloads); the DVE
      squares/multiplies double as the required fp32r rounding step.
    - phase order per output-half: gamma k-major across all 8 row-tiles (PE
      consumes the x/wm DMA stream in arrival order), gamma staged PSUM->SBUF,
      then delta m-major flowing into a pipelined epilogue.
"""

import numpy as np

N, IN, OUT = 8192, 1024, 1024
NCORES = 8
NS = N // NCORES          # 1024 rows of x per core
P = 128                   # SBUF partitions
KT = IN // P              # 8 contraction tiles
MT = NS // P              # 8 output-row tiles per core
OH = OUT // 2             # 512 = one PSUM bank of fp32
NHALF = 2

_CACHE = {}
LAST_RESULTS = None       # test harness can inspect exec_time_ns etc.


def _build_nc():
    import concourse.bass as bass
    import concourse.tile as tile
    from concourse import bacc, mybir

    f32 = mybir.dt.float32
    f32r = mybir.dt.float32r
    AFT = mybir.ActivationFunctionType

    nc = bacc.Bacc("TRN2", target_bir_lowering=False, debug=False)

    # Per-core DRAM tensors (host pre-transposed / pre-split layouts).
    # xT:   [IN, NS]        x shard, transposed (row = feature, col = sample)
    # wmT:  [2*IN, OH]      w_mean.T split into o-halves, stacked row-wise
    # wpT:  [2*IN, OH]      same for w_pho
    # zeta: [2*NS, OH]      zeta shard split into o-halves, stacked
    # out:  [2*NS, OH]      output, same split layout (host reassembles)
    # xT/wmT are declared float32r (same bytes as f32): the PE requires its
    # inputs "rounded to fp32r", and a same-dtype HWDGE DMA satisfies that
    # without a casting gpsimd DMA (which would serialize on one SW queue).
    xT = nc.dram_tensor("xT", [IN, NS], f32r, kind="ExternalInput").ap()
    wmT = nc.dram_tensor("wmT", [NHALF * IN, OH], f32r, kind="ExternalInput").ap()
    wpT = nc.dram_tensor("wpT", [NHALF * IN, OH], f32, kind="ExternalInput").ap()
    zeta = nc.dram_tensor("zeta", [NHALF * NS, OH], f32, kind="ExternalInput").ap()
    out = nc.dram_tensor("out", [NHALF * NS, OH], f32, kind="ExternalOutput").ap()

    with tile.TileContext(nc) as tc:
        with (
            tc.tile_pool(name="x", bufs=KT) as xp,
            tc.tile_pool(name="xsq", bufs=KT) as xsqp,
            tc.tile_pool(name="wm", bufs=NHALF * KT) as wmp,
            tc.tile_pool(name="wp", bufs=8) as wpp,
            tc.tile_pool(name="wv", bufs=NHALF * KT) as wvp,
            tc.tile_pool(name="gsb", bufs=KT) as gsbp,
            tc.tile_pool(name="z", bufs=4) as zp,
            tc.tile_pool(name="sd", bufs=3) as sdp,
            tc.tile_pool(name="t", bufs=3) as tp,
            tc.tile_pool(name="o", bufs=3) as op,
            tc.tile_pool(name="pg", bufs=4, space="PSUM") as pgp,
            tc.tile_pool(name="pd", bufs=4, space="PSUM") as pdp,
        ):
            x_t, xsq_t, wm_t, wv_t = [], [], {}, {}

            # Pre-load the one ACT LUT set that has Exp, Ln AND Square, so the
            # compiler's table-load pass (which greedily picks the *first*
            # table containing each function) doesn't thrash LUT loads when we
            # interleave exp/ln per tile. Sqrt later costs one more load.
            nc.scalar.add_instruction(mybir.InstLoadActFuncSet(
                name=nc.get_next_instruction_name(),
                act_func_set_id=6,  # natural_log_exp_and_others
                ins=[], outs=[]))

            # Main input stream on the SP HWDGE queue: x and the h=0 weight
            # half, interleaved per-k (the gamma k-loop consumes in this
            # order). wp rides the second (Activation-issued) HWDGE queue in
            # parallel; softplus = ln(1+exp(.)) in place, then a DVE square
            # that also rounds to fp32r.
            def prep_wvar(h, k):
                wpt = wpp.tile([P, OH], f32, tag="wp", name=f"wp{h}{k}")
                r0 = h * IN + k * P
                nc.scalar.dma_start(wpt[:], wpT[r0:r0 + P, :])
                nc.scalar.activation(wpt[:], wpt[:], AFT.Exp)
                nc.scalar.activation(wpt[:], wpt[:], AFT.Ln, bias=1.0)
                wvt = wvp.tile([P, OH], f32r, tag="wv", name=f"wv{h}{k}")
                nc.vector.tensor_mul(wvt[:], wpt[:], wpt[:])
                wv_t[h, k] = wvt

            for k in range(KT):
                xr = xp.tile([P, NS], f32r, tag="x", name=f"x{k}")
                nc.sync.dma_start(xr[:], xT[k * P:(k + 1) * P, :])
                x_t.append(xr)
                wmt = wmp.tile([P, OH], f32r, tag="wm", name=f"wm0{k}")
                nc.sync.dma_start(wmt[:], wmT[k * P:(k + 1) * P, :])
                wm_t[0, k] = wmt
                prep_wvar(0, k)
                xs = xsqp.tile([P, NS], f32r, tag="xsq", name=f"xsq{k}")
                nc.vector.tensor_mul(xs[:], xr[:], xr[:])
                xsq_t.append(xs)
            # wm1 rides the SP queue behind x/wm0; wp1 on the ACT queue.
            for k in range(KT):
                wmt = wmp.tile([P, OH], f32r, tag="wm", name=f"wm1{k}")
                nc.sync.dma_start(wmt[:], wmT[IN + k * P:IN + (k + 1) * P, :])
                wm_t[1, k] = wmt
            wp1 = []
            for k in range(KT):
                wpt = wpp.tile([P, OH], f32, tag="wp", name=f"wp1{k}")
                nc.scalar.dma_start(wpt[:], wpT[IN + k * P:IN + (k + 1) * P, :])
                wp1.append(wpt)

            def gamma_phase(h):
                # k-major in two 4-m sub-phases (4 PSUM banks each, staged to
                # SBUF between them) so the delta phase gets its own 4 banks
                # and its PSUM slots never serialize against gamma's.
                gsb = {}
                for sub in range(2):
                    mlo = sub * (MT // 2)
                    pgs = [pgp.tile([P, OH], f32, tag="pg", name=f"pg{h}{mlo + j}")
                           for j in range(MT // 2)]
                    for k in range(KT):
                        for j in range(MT // 2):
                            ms = slice((mlo + j) * P, (mlo + j + 1) * P)
                            nc.tensor.matmul(
                                pgs[j][:],
                                x_t[k][:, ms],
                                wm_t[h, k][:],
                                start=(k == 0),
                                stop=(k == KT - 1),
                            )
                    for j in range(MT // 2):
                        m = mlo + j
                        g = gsbp.tile([P, OH], f32, tag="gsb", name=f"gsb{h}{m}")
                        nc.vector.tensor_copy(g[:], pgs[j][:])
                        gsb[m] = g
                return gsb

            def delta_phase(h, gsb, interleave_prep1=False):
                # m-major: stage gamma[m] out of PSUM (freeing its bank for
                # the rolling delta accumulations), run m's delta k-loop, and
                # flow straight into the epilogue while the next m computes.
                # During h=0 the h=1 softplus is woven in one k per m, so the
                # in-order ACT stream never blocks on a long prep batch.
                for m in range(MT):
                    r0 = h * NS + m * P
                    zt = zp.tile([P, OH], f32, tag="z", name=f"z{h}{m}")
                    nc.scalar.dma_start(zt[:], zeta[r0:r0 + P, :])
                    pd = pdp.tile([P, OH], f32, tag="pd", name=f"pd{h}{m}")
                    ms = slice(m * P, (m + 1) * P)
                    for k in range(KT):
                        nc.tensor.matmul(
                            pd[:],
                            xsq_t[k][:, ms],
                            wv_t[h, k][:],
                            start=(k == 0),
                            stop=(k == KT - 1),
                        )
                    if interleave_prep1:
                        wpt = wp1[m]
                        nc.scalar.activation(wpt[:], wpt[:], AFT.Exp)
                        nc.scalar.activation(wpt[:], wpt[:], AFT.Ln, bias=1.0)
                        wvt = wvp.tile([P, OH], f32r, tag="wv", name=f"wv1{m}")
                        nc.vector.tensor_mul(wvt[:], wpt[:], wpt[:])
                        wv_t[1, m] = wvt
                    sd = sdp.tile([P, OH], f32, tag="sd", name=f"sd{h}{m}")
                    # sqrt(d) = exp(0.5*ln(d)): keeps every ACT op in the one
                    # preloaded LUT set (a real Sqrt would swap tables per m)
                    nc.scalar.activation(sd[:], pd[:], AFT.Ln)
                    nc.scalar.activation(sd[:], sd[:], AFT.Exp, scale=0.5)
                    tt = tp.tile([P, OH], f32, tag="t", name=f"t{h}{m}")
                    nc.vector.tensor_mul(tt[:], sd[:], zt[:])
                    ot = op.tile([P, OH], f32, tag="o", name=f"o{h}{m}")
                    nc.vector.tensor_add(ot[:], tt[:], gsb[m][:])
                    nc.scalar.dma_start(out[r0:r0 + P, :], ot[:])

            gsb0 = gamma_phase(0)
            delta_phase(0, gsb0)
            # h=1 softplus here: its ACT ops land in the window where the PE
            # is busy with gamma-h1 matmuls and the ACT stream is otherwise
            # idle (delta-h0's sqrt pairs precede, delta-h1's follow).
            for k in range(KT):
                wpt = wp1[k]
                nc.scalar.activation(wpt[:], wpt[:], AFT.Exp)
                nc.scalar.activation(wpt[:], wpt[:], AFT.Ln, bias=1.0)
                wvt = wvp.tile([P, OH], f32r, tag="wv", name=f"wv1{k}")
                nc.vector.tensor_mul(wvt[:], wpt[:], wpt[:])
                wv_t[1, k] = wvt
            gsb1 = gamma_phase(1)
            delta_phase(1, gsb1)

    nc.compile()
    return nc


def _get_nc():
    if "nc" not in _CACHE:
        _CACHE["nc"] = _build_nc()
    return _CACHE["nc"]


def _zeta_full():
    if "zeta" not in _CACHE:
        import jax
        import jax.numpy as jnp

        cpu = jax.devices("cpu")[0]
        with jax.default_device(cpu):
            z = jax.random.normal(jax.random.key(42), (N, OUT), dtype=jnp.float32)
            _CACHE["zeta"] = np.asarray(z)
    return _CACHE["zeta"]


def _split_halves(a):
    """[R, OUT] -> [2*R, OUT//2] with the two o-halves stacked row-wise."""
    return np.ascontiguousarray(np.concatenate([a[:, :OH], a[:, OH:]], axis=0))


def kernel(x, w_mean, w_pho):
    from concourse.bass_utils import run_bass_kernel_spmd

    global LAST_RESULTS

    x = np.ascontiguousarray(np.asarray(x, dtype=np.float32))
    w_mean = np.asarray(w_mean, dtype=np.float32)
    w_pho = np.asarray(w_pho, dtype=np.float32)

    nc = _get_nc()
    zeta = _zeta_full()

    xT_full = np.ascontiguousarray(x.T)                  # [IN, N]
    wmT = _split_halves(np.ascontiguousarray(w_mean.T))  # [2*IN, OH]
    wpT = _split_halves(np.ascontiguousarray(w_pho.T))

    in_maps = []
    for c in range(NCORES):
        cols = slice(c * NS, (c + 1) * NS)
        in_maps.append({
            "xT": np.ascontiguousarray(xT_full[:, cols]),
            "wmT": wmT,
            "wpT": wpT,
            "zeta": _split_halves(zeta[cols, :]),
        })

    try:
        res = run_bass_kernel_spmd(nc, in_maps, core_ids=list(range(NCORES)))
    except ModuleNotFoundError:
        # BASS_TRACE requested but this axon client has no NTFF profile hook
        # (antenv.axon_hooks missing) — rerun untraced.
        import os
        os.environ["BASS_NEVER_TRACE"] = "1"
        res = run_bass_kernel_spmd(nc, in_maps, core_ids=list(range(NCORES)))
    LAST_RESULTS = res

    out = np.empty((N, OUT), dtype=np.float32)
    for c in range(NCORES):
        o = res.results[c]["out"]                        # [2*NS, OH]
        out[c * NS:(c + 1) * NS, :OH] = o[:NS]
        out[c * NS:(c + 1) * NS, OH:] = o[NS:]
    return out
